# revision 22
# baseline (speedup 1.0000x reference)
"""Multi-head cross-modal attention + residual + LayerNorm on 8 TRN2 cores.

Reference computation (per batch b):
  Q = query @ Wq.T + bq ; K = key @ Wk.T + bk ; V = value @ Wv.T + bv
  attn = softmax(Q K^T / sqrt(D)) per head
  out  = (attn V) @ Wo.T + bo
  y    = LayerNorm(out + query) * gamma + beta

Sharding: 2-D over (batch=4) x (head-group=2). Core c owns batch c//2 and
heads [8*(c%2), 8*(c%2)+8) i.e. a 512-wide slice of the embedding dim for
Q/K/V/ctx. The out-projection over the 512-slice yields partial sums that a
pairwise ReduceScatter (groups [0,1],[2,3],[4,5],[6,7]) combines; each core
then applies residual+LayerNorm to its 512 rows and the host concatenates
the 8 [512,1024] results.

Precision strategy (validated numerically against the fp32 reference):
 - All projections (V/Q/K/out) and the ctx (attnV) matmul run as fp8e4m3
   with MatmulPerfMode.DoubleRow (2 contraction rows per PE pass). Weights
   are prescaled x32 on host so w~N(0,1/32^2) lands in fp8 normal range;
   scales are unwound exactly at evacuation points.
 - Scores (QK^T) stay bf16: fp8 Q/K noise passes through exp and is the
   one place precision is load-bearing.
 - exp uses a shift: E = exp(s - 4.25) so E fits fp8e4m3 (max score ~9.0,
   e4m3 max 240). Both the attnV numerator and the ones-column denominator
   use the same shifted weights, so the shift cancels in the softmax ratio.
 - A fraction of exp tiles run off the ACT engine as a bit-trick exp2: the
   fp8e4m3 bit pattern of 2^t is trunc(8*t + 56 + .5); a DVE affine
   (mult+add, PSUM source) then a Pool (max,0)+int8-convert (SBUF only:
   GPSIMD cannot touch PSUM) produce it. Max relative weight error ~3%,
   which softmax tolerates.
 - The out-proj partial sums cross cores as fp8 (x16 prescale to stay in
   normal range); residual+LayerNorm run fp32.

Engine budget: PE ~93us (DoubleRow), ACT = exp majority + LN rstd,
DVE = PSUM evacuations + fast-exp affines + reciprocal/bn, Pool =
ReduceScatter + fast-exp converts + LN elementwise.

Measured end-to-end error vs the fp32 reference: ~1e-2 of output absmax
(threshold 2e-2).
"""

import math
import sys

if "/opt/trn_rl_repo" not in sys.path:
    sys.path.insert(0, "/opt/trn_rl_repo")

import ml_dtypes
import numpy as np

import concourse.bass as bass  # noqa: F401  (registers types)
import concourse.mybir as mybir
import concourse.tile as tile
from concourse import bacc
from concourse.bass_utils import run_bass_kernel_spmd

F32 = mybir.dt.float32
F32R = mybir.dt.float32r
BF16 = mybir.dt.bfloat16
F8 = mybir.dt.float8e4
I8 = mybir.dt.int8
AF = mybir.ActivationFunctionType
OP = mybir.AluOpType
DR = mybir.MatmulPerfMode.DoubleRow

B, SQ, SK, E, H, D = 4, 1024, 2048, 1024, 16, 64
N_CORES = 8
OS = 512          # per-core slice of the embedding dim (8 heads x 64)
HL = 8            # local heads per core
ROWS = SQ // 2    # sequence rows each core owns after ReduceScatter
EPS = 1e-5

WS = 32.0                 # host-side weight prescale
C_SHIFT = 4.25            # exp shift: E = exp(s - C_SHIFT)
EXP_SCALE = 1.0 / (WS * WS * 8.0)          # raw PSUM score -> true score
FE_A = 8.0 * math.log2(math.e) * EXP_SCALE  # fast-exp affine mult
FE_B = 56.5 - 8.0 * math.log2(math.e) * C_SHIFT  # fast-exp affine add
OUT_SCALE = 16.0 / (WS * WS)  # po (1024*out_true) -> stage (16*out_true)

# knobs
DR_N = 512        # max out-free columns per DoubleRow matmul
RS_FP8 = True     # fp8 ReduceScatter payload
RB_DIRECT = True  # scalar_tensor_tensor in1 straight from PSUM (skip rb copy)
# per (head, jt) exp engine: 'A' = ACT exp, 'D' = DVE affine + Pool max/cvt
DVE_JT_LATE = {1, 3, 5, 7, 9, 11, 13}
EXP_SCHED = [
    ["D" if (h >= 2 and jt in DVE_JT_LATE) else "A" for jt in range(16)]
    for h in range(HL)
]

# module-level knobs used by test.py (harness ignores them)
TRACE = False
LAST_RESULTS = None

_NC_CACHE = None


def _dr_matmul(nc, out, lhsT, rhs3, start, stop):
    """DoubleRow matmul with the moving tensor chunked to DR_N out columns.

    out: PSUM AP [M, N]; lhsT: [128, 2, M] fp8; rhs3: [128, 2, N] fp8.
    """
    n_tot = rhs3.shape[-1]
    for n0 in range(0, n_tot, DR_N):
        n1 = min(n0 + DR_N, n_tot)
        nc.tensor.matmul(
            out[:, n0:n1],
            lhsT,
            rhs3[:, :, n0:n1],
            start=start,
            stop=stop,
            perf_mode=DR,
        )


def _build_nc():
    nc = bacc.Bacc(None, target_bir_lowering=False)

    qT8 = nc.dram_tensor("qT8", [512, 2 * SQ], F8, kind="ExternalInput")
    kT8 = nc.dram_tensor("kT8", [512, 2 * SK], F8, kind="ExternalInput")
    vT8 = nc.dram_tensor("vT8", [512, 2 * SK], F8, kind="ExternalInput")
    wq8 = nc.dram_tensor("wq8", [512, 2 * OS], F8, kind="ExternalInput")
    wk8 = nc.dram_tensor("wk8", [512, 2 * OS], F8, kind="ExternalInput")
    wv8 = nc.dram_tensor("wv8", [512, 2 * OS], F8, kind="ExternalInput")
    wo8 = nc.dram_tensor("wo8", [256, 2 * E], F8, kind="ExternalInput")
    bqk8 = nc.dram_tensor("bqk8", [128, 8], F32, kind="ExternalInput")
    resid = nc.dram_tensor("resid", [ROWS, E], F32, kind="ExternalInput")
    vec3 = nc.dram_tensor("vec3", [2, E], F32, kind="ExternalInput")
    ones64 = nc.dram_tensor("ones64", [1, 64], F32, kind="ExternalInput")
    out = nc.dram_tensor("out", [ROWS, E], F32, kind="ExternalOutput")

    rs_dt = F8 if RS_FP8 else BF16

    from contextlib import ExitStack

    with ExitStack() as ctx:
        tc = ctx.enter_context(tile.TileContext(nc))
        constp = ctx.enter_context(tc.tile_pool(name="consts", bufs=1))
        wp = ctx.enter_context(tc.tile_pool(name="wp", bufs=12))
        actp = ctx.enter_context(tc.tile_pool(name="actp", bufs=16))
        qtp = ctx.enter_context(tc.tile_pool(name="qtp", bufs=4))
        ktp = ctx.enter_context(tc.tile_pool(name="ktp", bufs=4))
        vsb = ctx.enter_context(tc.tile_pool(name="vsb", bufs=8))
        expp = ctx.enter_context(tc.tile_pool(name="expp", bufs=3))
        tmpp = ctx.enter_context(tc.tile_pool(name="tmpp", bufs=2))
        ctxp = ctx.enter_context(tc.tile_pool(name="ctxp", bufs=2))
        stp = ctx.enter_context(tc.tile_pool(name="stp", bufs=2))
        lnp = ctx.enter_context(tc.tile_pool(name="lnp", bufs=2))
        rbp = ctx.enter_context(tc.tile_pool(name="rbp", bufs=2))
        psc = ctx.enter_context(tc.tile_pool(name="psc", bufs=3, space="PSUM"))
        pc = ctx.enter_context(tc.tile_pool(name="pc", bufs=2, space="PSUM"))
        dramp = ctx.enter_context(tc.tile_pool(name="dramp", bufs=1, space="DRAM"))
        if True:
            eps_t = constp.tile([128, 1], F32)
            nc.vector.memset(eps_t, EPS)
            nbias_t = constp.tile([128, 1], F32)
            nc.vector.memset(nbias_t, -C_SHIFT)

            # small constants first (cheap, unblock Q/K evacuations)
            ones_r = constp.tile([1, 64], F32R)
            nc.sync.dma_start(out=ones_r, in_=ones64[:].bitcast(F32R))
            bqk_sb = constp.tile([128, 8], F32)
            nc.sync.dma_start(out=bqk_sb, in_=bqk8[:])
            bq_sb = bqk_sb[:, 0:4]
            bk_sb = bqk_sb[:, 4:8]

            # ---------------- Q projection: QT[o, i] (bf16, x32) -----------
            qt_dr = []
            wq_t = []
            for g in range(4):
                w = wp.tile([128, 2, OS], F8, tag="w", name=f"wq_{g}")
                nc.sync.dma_start(
                    out=w,
                    in_=wq8[g * 128 : (g + 1) * 128, :].rearrange(
                        "p (s n) -> p s n", s=2
                    ),
                )
                wq_t.append(w)
                a = actp.tile([128, 2, SQ], F8, tag="act", name=f"qin_{g}")
                nc.sync.dma_start(
                    out=a,
                    in_=qT8[g * 128 : (g + 1) * 128, :].rearrange(
                        "p (s n) -> p s n", s=2
                    ),
                )
                qt_dr.append(a)
            QTt = [
                qtp.tile([128, SQ], BF16, tag="qt", name=f"QT_{ot}")
                for ot in range(4)
            ]
            for ot in range(4):
                p = psc.tile([128, 1024], F32, tag="sc", name=f"pq_{ot}")
                for ih in range(2):
                    for g in range(4):
                        _dr_matmul(
                            nc,
                            p[:, ih * 512 : (ih + 1) * 512],
                            wq_t[g][:, :, ot * 128 : (ot + 1) * 128],
                            qt_dr[g][:, :, ih * 512 : (ih + 1) * 512],
                            start=(g == 0),
                            stop=(g == 3),
                        )
                nc.scalar.activation(
                    out=QTt[ot][:],
                    in_=p[:],
                    func=AF.Identity,
                    bias=bq_sb[:, ot : ot + 1],
                    scale=1.0,
                )

            # --- K loads + first K o-tile (rest pipelined into heads) ----
            wk_t = []
            kt_dr = []
            for g in range(4):
                w = wp.tile([128, 2, OS], F8, tag="w", name=f"wk_{g}")
                nc.sync.dma_start(
                    out=w,
                    in_=wk8[g * 128 : (g + 1) * 128, :].rearrange(
                        "p (s n) -> p s n", s=2
                    ),
                )
                wk_t.append(w)
                a = actp.tile([128, 2, SK], F8, tag="act", name=f"kin_{g}")
                kin3 = kT8[g * 128 : (g + 1) * 128, :].rearrange(
                    "p (s n) -> p s n", s=2
                )
                nc.sync.dma_start(out=a[:, :, 0:1024], in_=kin3[:, :, 0:1024])
                nc.sync.dma_start(out=a[:, :, 1024:SK], in_=kin3[:, :, 1024:SK])
                kt_dr.append(a)
            KTt = [
                ktp.tile([128, SK], BF16, tag="kt", name=f"KT_{ot}")
                for ot in range(4)
            ]

            def emit_k_groups(t4, groups, on_act=False):
                for jc in groups:
                    pf = psc.tile([128, 1024], F32, tag="sc", name=f"pk_{t4}_{jc}")
                    p = pf[:, 0:512]
                    for g in range(4):
                        _dr_matmul(
                            nc,
                            p[:],
                            wk_t[g][:, :, t4 * 128 : (t4 + 1) * 128],
                            kt_dr[g][:, :, jc * 512 : (jc + 1) * 512],
                            start=(g == 0),
                            stop=(g == 3),
                        )
                    if on_act:
                        nc.scalar.activation(
                            out=KTt[t4][:, jc * 512 : (jc + 1) * 512],
                            in_=p[:],
                            func=AF.Identity,
                            bias=bk_sb[:, t4 : t4 + 1],
                            scale=1.0,
                        )
                    else:
                        nc.vector.tensor_scalar_add(
                            out=KTt[t4][:, jc * 512 : (jc + 1) * 512],
                            in0=p[:],
                            scalar1=bk_sb[:, t4 : t4 + 1],
                        )

            emit_k_groups(0, range(4), on_act=True)

            # -------- V loads; projection is pipelined into heads 0-1 ------
            wv_t = []
            vt_dr = []
            for g in range(4):
                w = wp.tile([128, 2, OS], F8, tag="w", name=f"wv_{g}")
                nc.sync.dma_start(
                    out=w,
                    in_=wv8[g * 128 : (g + 1) * 128, :].rearrange(
                        "p (s n) -> p s n", s=2
                    ),
                )
                wv_t.append(w)
                a = actp.tile([128, 2, SK], F8, tag="act", name=f"vin_{g}")
                nc.sync.dma_start(
                    out=a,
                    in_=vT8[g * 128 : (g + 1) * 128, :].rearrange(
                        "p (s n) -> p s n", s=2
                    ),
                )
                vt_dr.append(a)
            # per-head stationary padded to 128 cols (64 V + ones + 63 zero):
            # DoubleRow ldweights rejects odd 65-wide stationaries, and the
            # matmul cost is charged on the moving dim so the pad is free.
            v_tiles = [
                vsb.tile([128, 2, HL * 128], F8, tag="v", name=f"V_{jtp}")
                for jtp in range(8)
            ]
            for jtp in range(8):
                vv = v_tiles[jtp].rearrange("p s (h c) -> p s h c", h=HL)
                nc.gpsimd.memset(vv[:, :, :, 64:65], 1.0)
                nc.gpsimd.memset(vv[:, :, :, 65:128], 0.0)

            def emit_v_pair(jtp):
                pv = psc.tile([128, 1024], F32, tag="sc", name=f"pv_{jtp}")
                for k in range(2):
                    jt = jtp * 2 + k
                    for g in range(4):
                        _dr_matmul(
                            nc,
                            pv[:, k * 512 : (k + 1) * 512],
                            vt_dr[g][:, :, jt * 128 : (jt + 1) * 128],
                            wv_t[g][:],
                            start=(g == 0),
                            stop=(g == 3),
                        )
                vv = v_tiles[jtp].rearrange("p s (h c) -> p s h c", h=HL)
                nc.vector.tensor_copy(
                    out=vv[:, :, :, 0:64],
                    in_=pv[:].rearrange("p (s h c) -> p s h c", s=2, h=HL),
                )

            emit_v_pair(0)
            emit_v_pair(1)

            # late-loaded constants (needed only at out-proj / LN)
            wo_t = []
            for g in range(2):
                w = wp.tile([128, 2, E], F8, tag="wo", name=f"wo_{g}")
                nc.sync.dma_start(
                    out=w,
                    in_=wo8[g * 128 : (g + 1) * 128, :].rearrange(
                        "p (s n) -> p s n", s=2
                    ),
                )
                wo_t.append(w)
            gamma_b = constp.tile([128, E], F32)
            nc.sync.dma_start(out=gamma_b, in_=vec3[0, :].partition_broadcast(128))
            beta_b = constp.tile([128, E], F32)
            nc.sync.dma_start(out=beta_b, in_=vec3[1, :].partition_broadcast(128))
            res_t = []
            for it in range(4):
                r = actp.tile([128, E], F32, tag="act", name=f"res_{it}")
                nc.sync.dma_start(
                    out=r, in_=resid[it * 128 : (it + 1) * 128, :]
                )
                res_t.append(r)

            ctx8 = [
                ctxp.tile([128, 2, SQ], F8, tag="ctx", name=f"ctx8_{g}")
                for g in range(2)
            ]

            def emit_head(h, fill=None):
                qt_tile = QTt[h // 2]
                kt_tile = KTt[h // 2]
                r0 = 64 * (h % 2)
                pcs = [
                    pc.tile([128, 512], F32, tag="ctx", name=f"pc_{h}_{ih}")
                    for ih in range(2)
                ]
                last = {}
                for jtp in range(8):
                    et = expp.tile([128, 2, SQ], F8, tag="exp", name=f"e_{h}_{jtp}")
                    for k in range(2):
                        jt = jtp * 2 + k
                        sp = psc.tile(
                            [128, 1024], F32, tag="sc", name=f"s_{h}_{jt}"
                        )
                        for ih in range(2):
                            nc.tensor.matmul(
                                sp[:, ih * 512 : (ih + 1) * 512],
                                kt_tile[r0 : r0 + 64, jt * 128 : (jt + 1) * 128],
                                qt_tile[r0 : r0 + 64, ih * 512 : (ih + 1) * 512],
                                start=True,
                                stop=True,
                            )
                        if EXP_SCHED[h][jt] == "A":
                            nc.scalar.activation(
                                out=et[:, k, :],
                                in_=sp[:],
                                func=AF.Exp,
                                scale=EXP_SCALE,
                                bias=nbias_t,
                            )
                        else:
                            tmp = tmpp.tile(
                                [128, 1024], F32, tag="tmp", name=f"t_{h}_{jt}"
                            )
                            nc.vector.tensor_scalar(
                                out=tmp,
                                in0=sp[:],
                                scalar1=FE_A,
                                scalar2=FE_B,
                                op0=OP.mult,
                                op1=OP.add,
                            )
                            nc.gpsimd.tensor_scalar_max(
                                out=et[:, k, :].bitcast(I8),
                                in0=tmp,
                                scalar1=0.0,
                            )
                    if fill is not None:
                        fill(jtp)

                    def emit_ctx(jtp=jtp, et=et):
                        for ih in range(2):
                            _dr_matmul(
                                nc,
                                pcs[ih][:],
                                v_tiles[jtp].rearrange(
                                    "p s (h c) -> p s h c", h=HL
                                )[:, :, h, :],
                                et[:, :, ih * 512 : (ih + 1) * 512],
                                start=(jtp == 0),
                                stop=(jtp == 7),
                            )

                    emit_ctx()

                # normalize closure: ctx8[d, i] = 32*ctx_true, fp8. Deferred
                # into the next head's first slot so the recip (DVE) finishes
                # behind that head's first scores instead of stalling the
                # in-order PE queue at the boundary.
                def norm(h=h, pcs=pcs):
                    g = h // 4
                    s = (h % 4) // 2
                    rb0 = 64 * (h % 2)
                    for ih in range(2):
                        rec = rbp.tile(
                            [1, 512], F32R, tag="rec", name=f"rc_{h}_{ih}"
                        )
                        with nc.allow_low_precision(reason="f32r fp32 bits"):
                            nc.vector.reciprocal(out=rec, in_=pcs[ih][64:65, :])
                        pbf = psc.tile(
                            [128, 1024], F32, tag="sc", name=f"pb_{h}_{ih}"
                        )
                        pbt = pbf[:, 0:512]
                        nc.tensor.matmul(
                            pbt[0:64, :], ones_r[:], rec[:], start=True, stop=True
                        )
                        rb = rbp.tile([64, 512], F32, tag="rb", name=f"rb_{h}_{ih}")
                        nc.vector.tensor_copy(out=rb, in_=pbt[0:64, :])
                        nc.vector.scalar_tensor_tensor(
                            out=ctx8[g][
                                rb0 : rb0 + 64, s, ih * 512 : (ih + 1) * 512
                            ],
                            in0=pcs[ih][0:64, :],
                            scalar=1.0,
                            in1=rb[:],
                            op0=OP.mult,
                            op1=OP.mult,
                        )

                return norm

            # software pipeline: V pairs 2-7 project during head 0; K o-tile
            # t+1 projects during the heads that consume KT[t]. ("V", j) =
            # emit_v_pair(j); ("K", t, jc) = emit_k_groups(t, [jc]).
            FILLS = {
                0: {1: [("V", 2)], 2: [("V", 3)], 3: [("V", 4)],
                    4: [("V", 5)], 5: [("V", 6)], 6: [("V", 7), ("K", 1, 0)],
                    7: [("K", 1, 1)]},
                1: {1: [("K", 1, 2)], 3: [("K", 1, 3)]},
                2: {2: [("K", 2, 0)], 4: [("K", 2, 1)]},
                3: {2: [("K", 2, 2)], 4: [("K", 2, 3)]},
                4: {2: [("K", 3, 0)], 4: [("K", 3, 1)]},
                5: {2: [("K", 3, 2)], 4: [("K", 3, 3)]},
            }

            def make_fill(h, prev_norm):
                plan = FILLS.get(h, {})

                def fill(jtp):
                    if jtp == 0 and prev_norm is not None:
                        prev_norm()
                    for item in plan.get(jtp, ()):
                        if item[0] == "V":
                            emit_v_pair(item[1])
                        else:
                            emit_k_groups(item[1], [item[2]])

                return fill

            prev_norm = None
            for h in range(HL):
                prev_norm = emit_head(h, fill=make_fill(h, prev_norm))
            prev_norm()

            # ---------------- out projection (partial sums) ----------------
            partial = dramp.tile([SQ, E], rs_dt, tag="partial")
            for it in range(8):
                stage = stp.tile([128, E], rs_dt, tag="stage", name=f"stage_{it}")
                po = psc.tile([128, 1024], F32, tag="sc", name=f"po_{it}")
                for eh in range(2):
                    for g in range(2):
                        _dr_matmul(
                            nc,
                            po[:, eh * 512 : (eh + 1) * 512],
                            ctx8[g][:, :, it * 128 : (it + 1) * 128],
                            wo_t[g][:, :, eh * 512 : (eh + 1) * 512],
                            start=(g == 0),
                            stop=(g == 1),
                        )
                nc.scalar.mul(out=stage[:], in_=po[:], mul=OUT_SCALE)
                nc.sync.dma_start(
                    out=partial[it * 128 : (it + 1) * 128, :], in_=stage
                )

            # ---------------- pairwise ReduceScatter ----------------
            rs_out = dramp.tile([ROWS, E], rs_dt, tag="rsout")
            nc.gpsimd.collective_compute(
                "ReduceScatter",
                OP.add,
                replica_groups=[[0, 1], [2, 3], [4, 5], [6, 7]],
                ins=[partial[:]],
                outs=[rs_out[:]],
            )

            # ---------------- residual + LayerNorm ----------------
            # resid already carries +bo (folded on host); x = rs/16 + resid
            xs, mvs, sds = [], [], []
            for it in range(4):
                x8 = lnp.tile([128, E], rs_dt, tag="x8", name=f"x8_{it}", bufs=4)
                nc.gpsimd.dma_start(
                    out=x8, in_=rs_out[it * 128 : (it + 1) * 128, :]
                )
                r = res_t[it]
                x = lnp.tile([128, E], F32, tag="x", name=f"x_{it}", bufs=4)
                nc.vector.scalar_tensor_tensor(
                    out=x,
                    in0=x8,
                    scalar=1.0 / 16.0,
                    in1=r,
                    op0=OP.mult,
                    op1=OP.add,
                )
                st = lnp.tile([128, 2, 6], F32, tag="st", name=f"st_{it}")
                xg = x.rearrange("p (g d) -> p g d", g=2)
                for sg in range(2):
                    nc.vector.bn_stats(out=st[:, sg, :], in_=xg[:, sg, :])
                mv = lnp.tile([128, 2], F32, tag="mv", name=f"mv_{it}", bufs=4)
                nc.vector.bn_aggr(out=mv, in_=st)
                xs.append(x)
                mvs.append(mv)
            # rstd = sqrt(1/(var+eps)): DVE recip + ACT Sqrt (keeps the ACT
            # func-set swaps to one: exp-table -> sqrt-table)
            for it in range(4):
                ve = lnp.tile([128, 1], F32, tag="ve", name=f"ve_{it}", bufs=4)
                nc.vector.tensor_scalar_add(
                    out=ve, in0=mvs[it][:, 1:2], scalar1=EPS
                )
                rv = lnp.tile([128, 1], F32, tag="rv", name=f"rv_{it}", bufs=4)
                with nc.allow_low_precision(reason="LN rstd tolerance"):
                    nc.vector.reciprocal(out=rv, in_=ve)
                sd = lnp.tile([128, 1], F32, tag="sd", name=f"sd_{it}", bufs=4)
                nc.scalar.activation(
                    out=sd, in_=rv, func=AF.Sqrt, bias=0.0, scale=1.0
                )
                sds.append(sd)
            for it in range(4):
                x = xs[it]
                nc.vector.tensor_scalar(
                    out=x,
                    in0=x,
                    scalar1=mvs[it][:, 0:1],
                    scalar2=sds[it],
                    op0=OP.subtract,
                    op1=OP.mult,
                )
                y = lnp.tile([128, E], F32, tag="y", name=f"y_{it}")
                nc.vector.scalar_tensor_tensor(
                    out=y,
                    in0=x,
                    scalar=1.0,
                    in1=gamma_b,
                    op0=OP.mult,
                    op1=OP.mult,
                )
                nc.gpsimd.tensor_add(out=y, in0=y, in1=beta_b)
                nc.sync.dma_start(
                    out=out[it * 128 : (it + 1) * 128, :], in_=y
                )

    nc.finalize()
    return nc


def _fp8(x):
    return np.clip(x, -240.0, 240.0).astype(ml_dtypes.float8_e4m3)


def _dr_pack(x):
    """[K, N] -> [K//256*128, 2*N]: row 256g+128s+p lands at [g*128+p, s*N+n]."""
    K, N = x.shape
    g = K // 256
    return np.ascontiguousarray(
        x.reshape(g, 2, 128, N).transpose(0, 2, 1, 3).reshape(g * 128, 2 * N)
    )


def build_in_maps(inputs):
    q = np.asarray(inputs["query"], dtype=np.float32)
    k = np.asarray(inputs["key"], dtype=np.float32)
    v = np.asarray(inputs["value"], dtype=np.float32)
    Wq = np.asarray(inputs["Wq"], dtype=np.float32)
    bq = np.asarray(inputs["bq"], dtype=np.float32)
    Wk = np.asarray(inputs["Wk"], dtype=np.float32)
    bk = np.asarray(inputs["bk"], dtype=np.float32)
    Wv = np.asarray(inputs["Wv"], dtype=np.float32)
    bv = np.asarray(inputs["bv"], dtype=np.float32)
    Wo = np.asarray(inputs["Wo"], dtype=np.float32)
    bo = np.asarray(inputs["bo"], dtype=np.float32)
    gamma = np.asarray(inputs["gamma"], dtype=np.float32)
    beta = np.asarray(inputs["beta"], dtype=np.float32)

    qT8 = [_dr_pack(_fp8(q[b].T)) for b in range(B)]
    kT8 = [_dr_pack(_fp8(k[b].T)) for b in range(B)]
    vT8 = [_dr_pack(_fp8(v[b].T)) for b in range(B)]

    # bv folded into a host-side bias vector: out includes +bv @ Wo.T + bo.
    bo_eff = (bv @ Wo.T + bo).astype(np.float32)
    ones32 = np.ones((1, 64), dtype=np.float32)

    in_maps = []
    for c in range(N_CORES):
        b, g = divmod(c, 2)
        sl = slice(OS * g, OS * g + OS)
        in_maps.append(
            {
                "qT8": qT8[b],
                "kT8": kT8[b],
                "vT8": vT8[b],
                "wq8": _dr_pack(_fp8(WS * Wq[sl, :].T)),
                "wk8": _dr_pack(_fp8(WS * Wk[sl, :].T)),
                "wv8": _dr_pack(_fp8(WS * Wv[sl, :].T)),
                "wo8": _dr_pack(_fp8(WS * Wo[:, sl].T)),
                "bqk8": np.ascontiguousarray(
                    WS
                    * np.concatenate(
                        [bq[sl].reshape(4, 128).T, bk[sl].reshape(4, 128).T], axis=1
                    )
                ),
                "resid": np.ascontiguousarray(
                    q[b, OS * g : OS * g + OS, :] + bo_eff
                ),
                "vec3": np.ascontiguousarray(np.stack([gamma, beta])),
                "ones64": ones32,
            }
        )
    return in_maps


def kernel(**inputs):
    global _NC_CACHE, LAST_RESULTS
    if _NC_CACHE is None:
        _NC_CACHE = _build_nc()
    nc = _NC_CACHE

    in_maps = build_in_maps(inputs)

    res = run_bass_kernel_spmd(nc, in_maps, list(range(N_CORES)), trace=TRACE)
    LAST_RESULTS = res

    outp = np.empty((B, SQ, E), dtype=np.float32)
    for c in range(N_CORES):
        b, g = divmod(c, 2)
        outp[b, OS * g : OS * g + OS, :] = res.results[c]["out"]
    return outp
